# revision 31
# baseline (speedup 1.0000x reference)
"""GCN message-passing kernel for Trainium2, 8-core SPMD (v2).

Model (N=8192 nodes, 64 graphs of 128 consecutive nodes):
  h   = emb[x]
  h   = GCN layer 1:  D_r^-1/2 m D_c^-1/2 relu(h W1^T + b1)
  h   = GCN layer 2:  D_r^-1/2 m D_c^-1/2 relu(h W2^T + b2)
  out = segment_max(h, 128-row blocks) @ Wc^T + bc

Distribution: row-shard m (1024 rows/core).

v2 design vs v1 baseline (564us -> target ~240us):
 - m loaded as f32 via HWDGE (sync) half-slabs at full HBM rate, PE-transposed,
   cast-copied to a resident bf16 mT[j%128, jt, i].  (v1 used a slow SWDGE
   casting DMA at ~139 GB/s.)
 - row degrees = free-axis DVE reduces on the f32 slabs; column-degree
   partials = chunked DVE reduces on mT, both overlapped with the load.
   (v1 had a 70us serial DVE reduce.)
 - ONE AllReduce for column degrees in [128, 64] p-major layout (v1: RS+AG
   with element-granular rearrange DMAs).
 - flipped aggregation: stationary = msg tile [j,128], moving = mT[j, i:512]
   -> 128 matmuls/layer with 512-wide moving operand, output t^T [F, i].
   Layer-2 stationary h1T comes free; segment-max becomes a free-axis
   reduce_max; no h1/h2 transposes.
 - msg2 allgathered UNscaled in [p, b, F] layout (clean 2KB/partition
   descriptors); the global column-degree scale s_c[p, jt] is applied after
   the AllGather, keeping the kernel rank-free.
"""

import sys

for p in ("/opt/trn_rl_repo",):
    if p not in sys.path:
        sys.path.insert(0, p)

from contextlib import ExitStack

import numpy as np

import concourse.bass as bass
import concourse.mybir as mybir
import concourse.tile as tile
from concourse import bacc, bass_utils
from concourse.masks import make_identity

P = 128
N = 8192
NCORES = 8
NS = N // NCORES          # rows per core (1024)
JT = N // P               # j tiles (64)
IB = NS // P              # i blocks per core (8)
F = 128                   # hidden/emb width
C = 16                    # classes
VOCAB = 32768
G_LOCAL = IB              # graphs per core (graph == one 128-row block)
QS = 2048                 # quarter-slab j width
JQ = QS // P              # j tiles per quarter-slab (16)
NQ = N // QS              # quarters per block row (4)

F32 = mybir.dt.float32
BF16 = mybir.dt.bfloat16
I32 = mybir.dt.int32

_CACHE = {}


def _build():
    nc = bacc.Bacc("TRN2", target_bir_lowering=False, debug=False,
                   enable_asserts=True, num_devices=NCORES)

    m_shard = nc.dram_tensor("m_shard", [NS, N], F32, kind="ExternalInput")
    x_in = nc.dram_tensor("x_in", [N], I32, kind="ExternalInput")
    emb_in = nc.dram_tensor("emb_in", [VOCAB, F], F32, kind="ExternalInput")
    w1_in = nc.dram_tensor("w1_in", [F, F], F32, kind="ExternalInput")
    b1_in = nc.dram_tensor("b1_in", [F], F32, kind="ExternalInput")
    w2_in = nc.dram_tensor("w2_in", [F, F], F32, kind="ExternalInput")
    b2_in = nc.dram_tensor("b2_in", [F], F32, kind="ExternalInput")
    wc_in = nc.dram_tensor("wc_in", [C, F], F32, kind="ExternalInput")
    bc_in = nc.dram_tensor("bc_in", [C], F32, kind="ExternalInput")
    out_l = nc.dram_tensor("out_l", [G_LOCAL, C], F32, kind="ExternalOutput")

    RELU = mybir.ActivationFunctionType.Relu
    COPY = mybir.ActivationFunctionType.Copy
    AX = mybir.AxisListType.X

    with tile.TileContext(nc) as tc, ExitStack() as stack:
        consts = stack.enter_context(tc.tile_pool(name="consts", bufs=1))
        big = stack.enter_context(tc.tile_pool(name="big", bufs=1))
        dram = stack.enter_context(tc.tile_pool(name="dram", bufs=1, space="DRAM"))

        ident_bf = consts.tile([P, P], BF16)
        make_identity(nc, ident_bf)
        ident_f32 = consts.tile([P, P], F32)
        make_identity(nc, ident_f32)

        # ---- small constants -------------------------------------------
        ones_row = consts.tile([1, P], BF16)
        nc.vector.memset(ones_row[:], 1.0)
        ones_colf = consts.tile([1, P], F32)
        nc.vector.memset(ones_colf[:], 1.0)
        ones_row8 = consts.tile([1, G_LOCAL], F32)
        nc.vector.memset(ones_row8[:], 1.0)
        b1_row = consts.tile([1, F], BF16)
        nc.gpsimd.dma_start(b1_row[:], b1_in.ap()[None, :])
        b2_row = consts.tile([1, F], BF16)
        nc.gpsimd.dma_start(b2_row[:], b2_in.ap()[None, :])
        bc_row = consts.tile([1, C], F32)
        nc.scalar.dma_start(bc_row[:], bc_in.ap()[None, :])
        x_sb = consts.tile([P, JT], I32)
        nc.gpsimd.dma_start(x_sb[:], x_in.ap().rearrange("(t p) -> p t", p=P))

        # w1T/w2T (transposed weights, bf16), wcT (f32)
        w1T = consts.tile([P, F], BF16)
        w2T = consts.tile([P, F], BF16)
        wcT = consts.tile([P, C], F32)
        with tc.tile_pool(name="wtmp", bufs=2) as wtmp, \
             tc.tile_pool(name="wpsum", bufs=2, space="PSUM") as wpsum:
            for w_in, wT in ((w1_in, w1T), (w2_in, w2T)):
                wf = wtmp.tile([F, F], F32, tag="wf")
                nc.scalar.dma_start(wf[:], w_in.ap())
                wb = wtmp.tile([F, F], BF16, tag="wb")
                nc.vector.tensor_copy(wb[:], wf[:])
                ps = wpsum.tile([P, F], BF16, tag="wps")
                nc.tensor.transpose(ps[:], wb[:], ident_bf[:])
                nc.vector.tensor_copy(wT[:], ps[:])
            wcf = wtmp.tile([C, F], F32, tag="wcf")
            nc.scalar.dma_start(wcf[:], wc_in.ap())
            pc = wpsum.tile([P, C], F32, tag="wcps")
            nc.tensor.transpose(pc[:], wcf[:], ident_f32[:C, :C])
            nc.vector.tensor_copy(wcT[:], pc[:])

        # ---- resident tensors ------------------------------------------
        # [j%128, i_block, jt, i%128] — contiguous per (block, jt-run) for
        # cheap PSUM copies and contiguous cd reduces
        mT = big.tile([P, IB, JT, P], BF16, tag="mT", name="mT")
        # hT holds h^T tiles during the load; each tile t is overwritten
        # in place with msg1' tile t right after its layer-1 msg matmul.
        hT = big.tile([P, JT, F], BF16, tag="hT", name="hT")      # [e, t, j%128]
        h1T = big.tile([P, NS], BF16, tag="h1T", name="h1T")      # [f, i]
        msg2_sb = big.tile([P, IB, F], BF16, tag="msg2", name="msg2")

        rd_parts = consts.tile([P, IB, NQ * 2], F32, tag="rd_parts",
                               name="rd_parts")
        rd_sb = consts.tile([P, IB], F32, tag="rd_sb", name="rd_sb")
        cd_part = consts.tile([P, JT], F32, tag="cd_part", name="cd_part")
        cd_acc = consts.tile([P, JT], F32, tag="cd_acc", name="cd_acc")
        cd_sb = consts.tile([P, JT], F32, tag="cd_sb", name="cd_sb")
        s_c = consts.tile([P, JT], F32, tag="s_c", name="s_c")
        srd_sb = consts.tile([P, IB], F32, tag="srd_sb", name="srd_sb")
        s_r_sb = consts.tile([P, IB], F32, tag="s_r_sb", name="s_r_sb")
        # per-block [1, 128] rows at partition 0 (matmul lhsT needs base 0)
        srd_row = [consts.tile([1, P], BF16, tag=f"srd_row{b}",
                               name=f"srd_row{b}") for b in range(IB)]
        srinv_row = [consts.tile([1, P], F32, tag=f"srinv_row{b}",
                                 name=f"srinv_row{b}") for b in range(IB)]

        # ---- phase L: m load (f32, HWDGE) -> engine cast to bf16 in
        # half-quarter chunks alternating vector/scalar (row-degree
        # partials fused via accum_out) -> bf16 PE transposes -> PSUM
        # copies into mT.  Column-degree partials: contiguous DVE reduce
        # per finished block.  Embedding gather pipeline runs
        # concurrently on gpsimd.
        MUL_OP = mybir.AluOpType.mult
        HQ = QS // 2  # half-quarter cast chunk (1024)
        with tc.tile_pool(name="slab", bufs=3) as slabp, \
             tc.tile_pool(name="slabb", bufs=3) as slabbp, \
             tc.tile_pool(name="tpsum", bufs=5, space="PSUM") as tpsum, \
             tc.tile_pool(name="hpsum", bufs=2, space="PSUM") as hpsum, \
             tc.tile_pool(name="hwork", bufs=5) as hwork:
            for b in range(IB):
                for qq in range(NQ):
                    hs = slabp.tile([P, QS], F32, tag="hs", name="hs")
                    nc.sync.dma_start(
                        hs[:], m_shard.ap()[b * P:(b + 1) * P,
                                            qq * QS:(qq + 1) * QS])
                    hsb = slabbp.tile([P, QS], BF16, tag="hsb", name="hsb")
                    for hf in range(2):
                        sl = slice(hf * HQ, (hf + 1) * HQ)
                        ri = qq * 2 + hf
                        # cast + fused row-degree partial
                        if hf == 0:
                            nc.vector.tensor_scalar(
                                out=hsb[:, sl], in0=hs[:, sl], scalar1=1.0,
                                scalar2=None, op0=MUL_OP,
                                op1=mybir.AluOpType.add,
                                accum_out=rd_parts[:, b, ri:ri + 1])
                        else:
                            nc.scalar.activation(
                                hsb[:, sl], hs[:, sl], COPY,
                                accum_out=rd_parts[:, b, ri:ri + 1])
                        for g in range(2):
                            ps = tpsum.tile([P, 4, P], BF16, tag="tp",
                                            name="tp")
                            for u in range(4):
                                jj = hf * 8 + g * 4 + u
                                nc.tensor.transpose(
                                    ps[:, u, :], hsb[:, jj * P:(jj + 1) * P],
                                    ident_bf[:])
                            jt0 = qq * JQ + hf * 8 + g * 4
                            if (hf * 2 + g) % 2 == 0:
                                nc.scalar.activation(
                                    mT[:, b, jt0:jt0 + 4, :], ps[:], COPY)
                            else:
                                nc.vector.tensor_copy(
                                    mT[:, b, jt0:jt0 + 4, :], ps[:])
                # column-degree partial for block b (contiguous reduce)
                if b == 0:
                    nc.vector.reduce_sum(
                        out=cd_acc[:], in_=mT[:, 0, :, :], axis=AX)
                else:
                    nc.vector.reduce_sum(
                        out=cd_part[:], in_=mT[:, b, :, :], axis=AX)
                    nc.vector.scalar_tensor_tensor(
                        out=cd_acc[:], in0=cd_acc[:], scalar=1.0,
                        in1=cd_part[:], op0=mybir.AluOpType.bypass,
                        op1=mybir.AluOpType.add)

            # embedding gather -> hT (gpsimd + vector cast + PE transpose)
            for t in range(JT):
                h_f = hwork.tile([P, F], F32, tag="hf", name="hf")
                nc.gpsimd.indirect_dma_start(
                    out=h_f[:],
                    out_offset=None,
                    in_=emb_in.ap(),
                    in_offset=bass.IndirectOffsetOnAxis(ap=x_sb[:, t:t + 1], axis=0),
                )
                h_b = hwork.tile([P, F], BF16, tag="hb", name="hb")
                nc.vector.tensor_copy(h_b[:], h_f[:])
                hp = hpsum.tile([P, P], BF16, tag="hp", name="hp")
                nc.tensor.transpose(hp[:], h_b[:], ident_bf[:])
                nc.scalar.activation(hT[:, t, :], hp[:], COPY)

        # ---- degrees: finalize + AllReduce -----------------------------
        BP = mybir.AluOpType.bypass
        MUL = mybir.AluOpType.mult
        # rd = sum over both halves
        nc.vector.reduce_sum(out=rd_sb[:], in_=rd_parts[:], axis=AX)

        cd_dram = dram.tile([P, JT], F32, tag="cd_dram", name="cd_dram")
        cd_out = dram.tile([P, JT], F32, tag="cd_out", name="cd_out",
                           addr_space="Shared")
        nc.sync.dma_start(cd_dram[:], cd_acc[:])
        nc.gpsimd.collective_compute(
            "AllReduce", mybir.AluOpType.add,
            replica_groups=[list(range(NCORES))],
            ins=[cd_dram.opt()], outs=[cd_out.opt()],
        )
        nc.sync.dma_start(cd_sb[:], cd_out[:])
        # NOTE: s_c = 1/sqrt(cd_sb) is emitted inside the layer-1 section,
        # after the AR-independent relu pass, so the scalar/vector FIFO
        # queues don't block on the AllReduce before doing useful work.

        # s_r (per-partition [p, b]) and row layouts for the bias trick
        nc.scalar.sqrt(srd_sb[:], rd_sb[:])
        nc.vector.reciprocal(s_r_sb[:], srd_sb[:])

        # flip srd/srinv columns into partition-0 rows via 1-col transposes
        with tc.tile_pool(name="rpsum", bufs=4, space="PSUM") as rpsum:
            for b in range(IB):
                ps1 = rpsum.tile([1, P], F32, tag="rps1", name="rps1")
                nc.tensor.transpose(ps1[:], srd_sb[:, b:b + 1], ident_f32[:])
                nc.vector.tensor_copy(srd_row[b][:], ps1[:])
                ps2 = rpsum.tile([1, P], F32, tag="rps2", name="rps2")
                nc.tensor.transpose(ps2[:], s_r_sb[:, b:b + 1], ident_f32[:])
                nc.vector.tensor_copy(srinv_row[b][:], ps2[:])

        # ---- layer 1: msg1' = relu(s_c * (h W1^T + b1)); t1T = mT'@msg -
        with tc.tile_pool(name="l1psum", bufs=1, space="PSUM") as l1p, \
             tc.tile_pool(name="mpsum", bufs=4, space="PSUM") as mp:
            t1 = [l1p.tile([P, 4, P], F32, tag=f"t1_{hh}", name=f"t1_{hh}")
                  for hh in range(2)]
            # part A (AR-independent): msg1 pre-activations, relu in place
            for t in range(JT):
                mps = mp.tile([P, F], F32, tag="mps", name="mps")
                nc.tensor.matmul(mps[:], hT[:, t, :], w1T[:],
                                 start=True, stop=False)
                nc.tensor.matmul(mps[:], ones_row[:], b1_row[:],
                                 start=False, stop=True)
                nc.scalar.activation(hT[:, t, :], mps[:], RELU)
            # s_c emitted here so the scalar/vector queues only block on
            # the AllReduce after finishing all relu work
            nc.scalar.sqrt(s_c[:], cd_sb[:])
            nc.vector.reciprocal(s_c[:], s_c[:])
            # part B: scale tiles by s_c and aggregate
            for t in range(JT):
                nc.vector.tensor_scalar_mul(
                    hT[:, t, :], hT[:, t, :], s_c[:, t:t + 1])
                for hh in range(2):
                    nc.tensor.matmul(
                        t1[hh][:], hT[:, t, :],
                        mT[:, hh * 4:(hh + 1) * 4, t, :],
                        start=(t == 0), stop=(t == JT - 1))
            # h1T = raw t1 (scales folded into the layer-2 msg step)
            for hh in range(2):
                nc.scalar.activation(h1T[:, hh * 512:(hh + 1) * 512],
                                     t1[hh][:], COPY)

            # ---- layer 2 messages (local blocks only) ------------------
            # u = t1 @ W2^T ; msg2 = relu(s_r*u + b2) = relu(h1@W2^T + b2)
            with tc.tile_pool(name="m2psum", bufs=2, space="PSUM") as m2p:
                for b in range(IB):
                    ps = m2p.tile([P, F], F32, tag="m2ps", name="m2ps")
                    nc.tensor.matmul(ps[:], h1T[:, b * P:(b + 1) * P], w2T[:],
                                     start=True, stop=False)
                    nc.tensor.matmul(ps[:], srd_row[b][:], b2_row[:],
                                     start=False, stop=True)
                    nc.scalar.activation(msg2_sb[:, b, :], ps[:], RELU,
                                         scale=s_r_sb[:, b:b + 1])

        # ---- msg2 AllGather (unscaled, [p, b, F] layout) ---------------
        msg2_loc = dram.tile([P, IB, F], BF16, tag="m2l", name="m2l")
        msg2_full = dram.tile([NCORES, P, IB, F], BF16, tag="m2f", name="m2f",
                              addr_space="Shared")
        nc.sync.dma_start(msg2_loc[:], msg2_sb[:])
        nc.gpsimd.collective_compute(
            "AllGather", mybir.AluOpType.bypass,
            replica_groups=[list(range(NCORES))],
            ins=[msg2_loc.opt()], outs=[msg2_full.opt()],
        )

        with tc.tile_pool(name="late", bufs=1) as late, \
             tc.tile_pool(name="l2psum", bufs=1, space="PSUM") as l2p, \
             tc.tile_pool(name="srpsum", bufs=2, space="PSUM") as srp:
            # srb[f, i] = s_r[i] broadcast across partitions (rank-1 matmuls)
            srb = late.tile([P, IB, P], F32, tag="srb", name="srb")
            for hh in range(2):
                sps = srp.tile([P, 4, P], F32, tag="sps", name="sps")
                for u in range(4):
                    b = hh * 4 + u
                    nc.tensor.matmul(sps[:, u, :], ones_colf[:],
                                     srinv_row[b][:],
                                     start=True, stop=True)
                nc.vector.tensor_copy(srb[:, hh * 4:(hh + 1) * 4, :], sps[:])

            msg2_all = late.tile([P, JT, F], BF16, tag="m2a", name="m2a")
            nc.sync.dma_start(
                msg2_all[:].rearrange("p (r b) g -> p r b g", r=NCORES),
                msg2_full[:].rearrange("r p b g -> p r b g"))

            # ---- layer 2 aggregation: t2T = sum_t (s_c*msg2)_t^T mT_t --
            t2 = [l2p.tile([P, 4, P], F32, tag=f"t2_{hh}", name=f"t2_{hh}")
                  for hh in range(2)]
            for t in range(JT):
                nc.vector.tensor_scalar_mul(
                    msg2_all[:, t, :], msg2_all[:, t, :], s_c[:, t:t + 1])
                for hh in range(2):
                    nc.tensor.matmul(
                        t2[hh][:], msg2_all[:, t, :],
                        mT[:, hh * 4:(hh + 1) * 4, t, :],
                        start=(t == 0), stop=(t == JT - 1))

            # ---- h2 = srb * t2T; segment max; classifier ---------------
            h2s = late.tile([P, IB, P], F32, tag="h2s", name="h2s")
            pooledT = late.tile([P, G_LOCAL], F32, tag="pooledT", name="pooledT")
            out_sb = late.tile([G_LOCAL, C], F32, tag="out_sb", name="out_sb")
            for hh in range(2):
                nc.vector.scalar_tensor_tensor(
                    out=h2s[:, hh * 4:(hh + 1) * 4, :], in0=t2[hh][:],
                    scalar=1.0, in1=srb[:, hh * 4:(hh + 1) * 4, :],
                    op0=BP, op1=MUL)
                nc.vector.reduce_max(
                    out=pooledT[:, hh * 4:(hh + 1) * 4],
                    in_=h2s[:, hh * 4:(hh + 1) * 4, :], axis=AX)
            with tc.tile_pool(name="clspsum", bufs=1, space="PSUM") as clsp:
                cps = clsp.tile([G_LOCAL, C], F32, tag="cls", name="cls")
                nc.tensor.matmul(cps[:], pooledT[:], wcT[:],
                                 start=True, stop=False)
                nc.tensor.matmul(cps[:], ones_row8[:], bc_row[:],
                                 start=False, stop=True)
                nc.vector.tensor_copy(out_sb[:], cps[:])
            nc.sync.dma_start(out_l.ap(), out_sb[:])

    nc.compile()
    return nc


def _get_nc():
    if "nc" not in _CACHE:
        _CACHE["nc"] = _build()
    return _CACHE["nc"]


def kernel(**inputs):
    m = np.ascontiguousarray(np.asarray(inputs["m"], dtype=np.float32))
    x = np.ascontiguousarray(np.asarray(inputs["x"]).astype(np.int32))
    emb = np.ascontiguousarray(np.asarray(inputs["emb"], dtype=np.float32))
    w1 = np.ascontiguousarray(np.asarray(inputs["w1"], dtype=np.float32))
    b1 = np.ascontiguousarray(np.asarray(inputs["b1"], dtype=np.float32))
    w2 = np.ascontiguousarray(np.asarray(inputs["w2"], dtype=np.float32))
    b2 = np.ascontiguousarray(np.asarray(inputs["b2"], dtype=np.float32))
    wc = np.ascontiguousarray(np.asarray(inputs["wc"], dtype=np.float32))
    bc = np.ascontiguousarray(np.asarray(inputs["bc"], dtype=np.float32))

    nc = _get_nc()
    in_maps = []
    for k in range(NCORES):
        in_maps.append({
            "m_shard": np.ascontiguousarray(m[k * NS:(k + 1) * NS]),
            "x_in": x, "emb_in": emb,
            "w1_in": w1, "b1_in": b1, "w2_in": w2, "b2_in": b2,
            "wc_in": wc, "bc_in": bc,
        })
    res = bass_utils.run_bass_kernel_spmd(
        nc, in_maps, core_ids=list(range(NCORES)))
    out = np.concatenate([res.results[k]["out_l"] for k in range(NCORES)], axis=0)
    return out.astype(np.float32)


# revision 34
# speedup vs baseline: 1.1667x; 1.1667x over previous
"""GCN message-passing kernel for Trainium2, 8-core SPMD (v2).

Model (N=8192 nodes, 64 graphs of 128 consecutive nodes):
  h   = emb[x]
  h   = GCN layer 1:  D_r^-1/2 m D_c^-1/2 relu(h W1^T + b1)
  h   = GCN layer 2:  D_r^-1/2 m D_c^-1/2 relu(h W2^T + b2)
  out = segment_max(h, 128-row blocks) @ Wc^T + bc

Distribution: row-shard m (1024 rows/core).

v2 design vs v1 baseline (564us -> target ~240us):
 - m loaded as f32 via HWDGE (sync) half-slabs at full HBM rate, PE-transposed,
   cast-copied to a resident bf16 mT[j%128, jt, i].  (v1 used a slow SWDGE
   casting DMA at ~139 GB/s.)
 - row degrees = free-axis DVE reduces on the f32 slabs; column-degree
   partials = chunked DVE reduces on mT, both overlapped with the load.
   (v1 had a 70us serial DVE reduce.)
 - ONE AllReduce for column degrees in [128, 64] p-major layout (v1: RS+AG
   with element-granular rearrange DMAs).
 - flipped aggregation: stationary = msg tile [j,128], moving = mT[j, i:512]
   -> 128 matmuls/layer with 512-wide moving operand, output t^T [F, i].
   Layer-2 stationary h1T comes free; segment-max becomes a free-axis
   reduce_max; no h1/h2 transposes.
 - msg2 allgathered UNscaled in [p, b, F] layout (clean 2KB/partition
   descriptors); the global column-degree scale s_c[p, jt] is applied after
   the AllGather, keeping the kernel rank-free.
"""

import sys

for p in ("/opt/trn_rl_repo",):
    if p not in sys.path:
        sys.path.insert(0, p)

from contextlib import ExitStack

import numpy as np

import concourse.bass as bass
import concourse.mybir as mybir
import concourse.tile as tile
from concourse import bacc, bass_utils
from concourse.masks import make_identity

P = 128
N = 8192
NCORES = 8
NS = N // NCORES          # rows per core (1024)
JT = N // P               # j tiles (64)
IB = NS // P              # i blocks per core (8)
F = 128                   # hidden/emb width
C = 16                    # classes
VOCAB = 32768
G_LOCAL = IB              # graphs per core (graph == one 128-row block)
QS = 2048                 # quarter-slab j width
JQ = QS // P              # j tiles per quarter-slab (16)
NQ = N // QS              # quarters per block row (4)

F32 = mybir.dt.float32
BF16 = mybir.dt.bfloat16
I32 = mybir.dt.int32

_CACHE = {}


def _build():
    nc = bacc.Bacc("TRN2", target_bir_lowering=False, debug=False,
                   enable_asserts=True, num_devices=NCORES)

    m_shard = nc.dram_tensor("m_shard", [NS, N], F32, kind="ExternalInput")
    x_in = nc.dram_tensor("x_in", [N], I32, kind="ExternalInput")
    emb_in = nc.dram_tensor("emb_in", [VOCAB, F], F32, kind="ExternalInput")
    w1_in = nc.dram_tensor("w1_in", [F, F], F32, kind="ExternalInput")
    b1_in = nc.dram_tensor("b1_in", [F], F32, kind="ExternalInput")
    w2_in = nc.dram_tensor("w2_in", [F, F], F32, kind="ExternalInput")
    b2_in = nc.dram_tensor("b2_in", [F], F32, kind="ExternalInput")
    wc_in = nc.dram_tensor("wc_in", [C, F], F32, kind="ExternalInput")
    bc_in = nc.dram_tensor("bc_in", [C], F32, kind="ExternalInput")
    out_l = nc.dram_tensor("out_l", [G_LOCAL, C], F32, kind="ExternalOutput")

    RELU = mybir.ActivationFunctionType.Relu
    COPY = mybir.ActivationFunctionType.Copy
    AX = mybir.AxisListType.X

    with tile.TileContext(nc) as tc, ExitStack() as stack:
        consts = stack.enter_context(tc.tile_pool(name="consts", bufs=1))
        big = stack.enter_context(tc.tile_pool(name="big", bufs=1))
        dram = stack.enter_context(tc.tile_pool(name="dram", bufs=1, space="DRAM"))

        ident_bf = consts.tile([P, P], BF16)
        make_identity(nc, ident_bf)
        ident_f32 = consts.tile([P, P], F32)
        make_identity(nc, ident_f32)

        # ---- small constants -------------------------------------------
        ones_row = consts.tile([1, P], BF16)
        nc.vector.memset(ones_row[:], 1.0)
        ones_colf = consts.tile([1, P], F32)
        nc.vector.memset(ones_colf[:], 1.0)
        ones_row8 = consts.tile([1, G_LOCAL], F32)
        nc.vector.memset(ones_row8[:], 1.0)
        b1_row = consts.tile([1, F], BF16)
        nc.gpsimd.dma_start(b1_row[:], b1_in.ap()[None, :])
        b2_row = consts.tile([1, F], BF16)
        nc.gpsimd.dma_start(b2_row[:], b2_in.ap()[None, :])
        bc_row = consts.tile([1, C], F32)
        nc.scalar.dma_start(bc_row[:], bc_in.ap()[None, :])
        x_sb = consts.tile([P, JT], I32)
        nc.gpsimd.dma_start(x_sb[:], x_in.ap().rearrange("(t p) -> p t", p=P))

        # w1T/w2T (transposed weights, bf16), wcT (f32)
        w1T = consts.tile([P, F], BF16)
        w2T = consts.tile([P, F], BF16)
        wcT = consts.tile([P, C], F32)
        with tc.tile_pool(name="wtmp", bufs=2) as wtmp, \
             tc.tile_pool(name="wpsum", bufs=2, space="PSUM") as wpsum:
            for w_in, wT in ((w1_in, w1T), (w2_in, w2T)):
                wf = wtmp.tile([F, F], F32, tag="wf")
                nc.scalar.dma_start(wf[:], w_in.ap())
                wb = wtmp.tile([F, F], BF16, tag="wb")
                nc.vector.tensor_copy(wb[:], wf[:])
                ps = wpsum.tile([P, F], BF16, tag="wps")
                nc.tensor.transpose(ps[:], wb[:], ident_bf[:])
                nc.vector.tensor_copy(wT[:], ps[:])
            wcf = wtmp.tile([C, F], F32, tag="wcf")
            nc.scalar.dma_start(wcf[:], wc_in.ap())
            pc = wpsum.tile([P, C], F32, tag="wcps")
            nc.tensor.transpose(pc[:], wcf[:], ident_f32[:C, :C])
            nc.vector.tensor_copy(wcT[:], pc[:])

        # ---- resident tensors ------------------------------------------
        # [j%128, i_block, jt, i%128] — contiguous per (block, jt-run) for
        # cheap PSUM copies and contiguous cd reduces
        mT = big.tile([P, IB, JT, P], BF16, tag="mT", name="mT")
        # hT holds h^T tiles during the load; each tile t is overwritten
        # in place with msg1' tile t right after its layer-1 msg matmul.
        hT = big.tile([P, JT, F], BF16, tag="hT", name="hT")      # [e, t, j%128]
        h1T = big.tile([P, NS], BF16, tag="h1T", name="h1T")      # [f, i]
        msg2_sb = big.tile([P, IB, F], BF16, tag="msg2", name="msg2")

        rd_parts = consts.tile([P, IB, NQ * 2], F32, tag="rd_parts",
                               name="rd_parts")
        rd_sb = consts.tile([P, IB], F32, tag="rd_sb", name="rd_sb")
        cd_part = consts.tile([P, JT], F32, tag="cd_part", name="cd_part")
        cd_acc = consts.tile([P, JT], F32, tag="cd_acc", name="cd_acc")
        cd_sb = consts.tile([P, JT], F32, tag="cd_sb", name="cd_sb")
        s_c = consts.tile([P, JT], F32, tag="s_c", name="s_c")
        srd_sb = consts.tile([P, IB], F32, tag="srd_sb", name="srd_sb")
        s_r_sb = consts.tile([P, IB], F32, tag="s_r_sb", name="s_r_sb")
        # per-block [1, 128] rows at partition 0 (matmul lhsT needs base 0)
        srd_row = [consts.tile([1, P], BF16, tag=f"srd_row{b}",
                               name=f"srd_row{b}") for b in range(IB)]
        srinv_row = [consts.tile([1, P], F32, tag=f"srinv_row{b}",
                                 name=f"srinv_row{b}") for b in range(IB)]

        # ---- phase L: m load (f32, HWDGE) -> engine cast to bf16 in
        # half-quarter chunks alternating vector/scalar (row-degree
        # partials fused via accum_out) -> bf16 PE transposes -> PSUM
        # copies into mT.  Column-degree partials: contiguous DVE reduce
        # per finished block.  Embedding gather pipeline runs
        # concurrently on gpsimd.
        MUL_OP = mybir.AluOpType.mult
        HQ = QS // 2  # half-quarter cast chunk (1024)
        with tc.tile_pool(name="slab", bufs=3) as slabp, \
             tc.tile_pool(name="slabb", bufs=3) as slabbp, \
             tc.tile_pool(name="tpsum", bufs=5, space="PSUM") as tpsum, \
             tc.tile_pool(name="hpsum", bufs=2, space="PSUM") as hpsum, \
             tc.tile_pool(name="hwork", bufs=5) as hwork:
            for b in range(IB):
                for qq in range(NQ):
                    hs = slabp.tile([P, QS], F32, tag="hs", name="hs")
                    nc.sync.dma_start(
                        hs[:], m_shard.ap()[b * P:(b + 1) * P,
                                            qq * QS:(qq + 1) * QS])
                    hsb = slabbp.tile([P, QS], BF16, tag="hsb", name="hsb")
                    for hf in range(2):
                        sl = slice(hf * HQ, (hf + 1) * HQ)
                        ri = qq * 2 + hf
                        # cast + fused row-degree partial
                        if hf == 0:
                            nc.vector.tensor_scalar(
                                out=hsb[:, sl], in0=hs[:, sl], scalar1=1.0,
                                scalar2=None, op0=MUL_OP,
                                op1=mybir.AluOpType.add,
                                accum_out=rd_parts[:, b, ri:ri + 1])
                        else:
                            nc.scalar.activation(
                                hsb[:, sl], hs[:, sl], COPY,
                                accum_out=rd_parts[:, b, ri:ri + 1])
                        for g in range(2):
                            ps = tpsum.tile([P, 4, P], BF16, tag="tp",
                                            name="tp")
                            for u in range(4):
                                jj = hf * 8 + g * 4 + u
                                nc.tensor.transpose(
                                    ps[:, u, :], hsb[:, jj * P:(jj + 1) * P],
                                    ident_bf[:])
                            jt0 = qq * JQ + hf * 8 + g * 4
                            nc.scalar.activation(
                                mT[:, b, jt0:jt0 + 4, :], ps[:], COPY)
                # column-degree partial for block b (contiguous reduce)
                if b == 0:
                    nc.vector.reduce_sum(
                        out=cd_acc[:], in_=mT[:, 0, :, :], axis=AX)
                else:
                    nc.vector.reduce_sum(
                        out=cd_part[:], in_=mT[:, b, :, :], axis=AX)
                    nc.vector.scalar_tensor_tensor(
                        out=cd_acc[:], in0=cd_acc[:], scalar=1.0,
                        in1=cd_part[:], op0=mybir.AluOpType.bypass,
                        op1=mybir.AluOpType.add)

            # embedding gather -> hT (gpsimd + vector cast + PE transpose)
            for t in range(JT):
                h_f = hwork.tile([P, F], F32, tag="hf", name="hf")
                nc.gpsimd.indirect_dma_start(
                    out=h_f[:],
                    out_offset=None,
                    in_=emb_in.ap(),
                    in_offset=bass.IndirectOffsetOnAxis(ap=x_sb[:, t:t + 1], axis=0),
                )
                h_b = hwork.tile([P, F], BF16, tag="hb", name="hb")
                nc.vector.tensor_copy(h_b[:], h_f[:])
                hp = hpsum.tile([P, P], BF16, tag="hp", name="hp")
                nc.tensor.transpose(hp[:], h_b[:], ident_bf[:])
                nc.scalar.activation(hT[:, t, :], hp[:], COPY)

        # ---- degrees: finalize + AllReduce -----------------------------
        BP = mybir.AluOpType.bypass
        MUL = mybir.AluOpType.mult
        # rd = sum over both halves
        nc.vector.reduce_sum(out=rd_sb[:], in_=rd_parts[:], axis=AX)

        cd_dram = dram.tile([P, JT], F32, tag="cd_dram", name="cd_dram")
        cd_out = dram.tile([P, JT], F32, tag="cd_out", name="cd_out",
                           addr_space="Shared")
        nc.sync.dma_start(cd_dram[:], cd_acc[:])
        nc.gpsimd.collective_compute(
            "AllReduce", mybir.AluOpType.add,
            replica_groups=[list(range(NCORES))],
            ins=[cd_dram.opt()], outs=[cd_out.opt()],
        )
        nc.sync.dma_start(cd_sb[:], cd_out[:])
        nc.scalar.sqrt(s_c[:], cd_sb[:])
        nc.vector.reciprocal(s_c[:], s_c[:])

        # s_r (per-partition [p, b]) and row layouts for the bias trick
        nc.scalar.sqrt(srd_sb[:], rd_sb[:])
        nc.vector.reciprocal(s_r_sb[:], srd_sb[:])

        # flip srd/srinv columns into partition-0 rows via 1-col transposes
        with tc.tile_pool(name="rpsum", bufs=4, space="PSUM") as rpsum:
            for b in range(IB):
                ps1 = rpsum.tile([1, P], F32, tag="rps1", name="rps1")
                nc.tensor.transpose(ps1[:], srd_sb[:, b:b + 1], ident_f32[:])
                nc.vector.tensor_copy(srd_row[b][:], ps1[:])
                ps2 = rpsum.tile([1, P], F32, tag="rps2", name="rps2")
                nc.tensor.transpose(ps2[:], s_r_sb[:, b:b + 1], ident_f32[:])
                nc.vector.tensor_copy(srinv_row[b][:], ps2[:])

        # ---- layer 1: msg1' = relu(s_c * (h W1^T + b1)); t1T = mT'@msg -
        with tc.tile_pool(name="l1psum", bufs=1, space="PSUM") as l1p, \
             tc.tile_pool(name="mpsum", bufs=4, space="PSUM") as mp:
            t1 = [l1p.tile([P, 4, P], F32, tag=f"t1_{hh}", name=f"t1_{hh}")
                  for hh in range(2)]
            for t in range(JT):
                mps = mp.tile([P, F], F32, tag="mps", name="mps")
                nc.tensor.matmul(mps[:], hT[:, t, :], w1T[:],
                                 start=True, stop=False)
                nc.tensor.matmul(mps[:], ones_row[:], b1_row[:],
                                 start=False, stop=True)
                # overwrite hT tile t (now dead) with relu(msg1) tile t;
                # the s_c scale is applied separately on DVE so the relu
                # (and the msg matmuls) can overlap the cd AllReduce.
                nc.scalar.activation(hT[:, t, :], mps[:], RELU)
                nc.vector.tensor_scalar_mul(
                    hT[:, t, :], hT[:, t, :], s_c[:, t:t + 1])
                for hh in range(2):
                    nc.tensor.matmul(
                        t1[hh][:], hT[:, t, :],
                        mT[:, hh * 4:(hh + 1) * 4, t, :],
                        start=(t == 0), stop=(t == JT - 1))
            # h1T = raw t1 (scales folded into the layer-2 msg step)
            for hh in range(2):
                nc.scalar.activation(h1T[:, hh * 512:(hh + 1) * 512],
                                     t1[hh][:], COPY)

            # ---- layer 2 messages (local blocks only) ------------------
            # u = t1 @ W2^T ; msg2 = relu(s_r*u + b2) = relu(h1@W2^T + b2)
            with tc.tile_pool(name="m2psum", bufs=2, space="PSUM") as m2p:
                for b in range(IB):
                    ps = m2p.tile([P, F], F32, tag="m2ps", name="m2ps")
                    nc.tensor.matmul(ps[:], h1T[:, b * P:(b + 1) * P], w2T[:],
                                     start=True, stop=False)
                    nc.tensor.matmul(ps[:], srd_row[b][:], b2_row[:],
                                     start=False, stop=True)
                    nc.scalar.activation(msg2_sb[:, b, :], ps[:], RELU,
                                         scale=s_r_sb[:, b:b + 1])

        # ---- msg2 AllGather (unscaled, [p, b, F] layout) ---------------
        msg2_loc = dram.tile([P, IB, F], BF16, tag="m2l", name="m2l")
        msg2_full = dram.tile([NCORES, P, IB, F], BF16, tag="m2f", name="m2f",
                              addr_space="Shared")
        nc.sync.dma_start(msg2_loc[:], msg2_sb[:])
        nc.gpsimd.collective_compute(
            "AllGather", mybir.AluOpType.bypass,
            replica_groups=[list(range(NCORES))],
            ins=[msg2_loc.opt()], outs=[msg2_full.opt()],
        )

        with tc.tile_pool(name="late", bufs=1) as late, \
             tc.tile_pool(name="l2psum", bufs=1, space="PSUM") as l2p, \
             tc.tile_pool(name="srpsum", bufs=2, space="PSUM") as srp:
            # srb[f, i] = s_r[i] broadcast across partitions (rank-1 matmuls)
            srb = late.tile([P, IB, P], F32, tag="srb", name="srb")
            for hh in range(2):
                sps = srp.tile([P, 4, P], F32, tag="sps", name="sps")
                for u in range(4):
                    b = hh * 4 + u
                    nc.tensor.matmul(sps[:, u, :], ones_colf[:],
                                     srinv_row[b][:],
                                     start=True, stop=True)
                nc.vector.tensor_copy(srb[:, hh * 4:(hh + 1) * 4, :], sps[:])

            msg2_all = late.tile([P, JT, F], BF16, tag="m2a", name="m2a")
            nc.sync.dma_start(
                msg2_all[:].rearrange("p (r b) g -> p r b g", r=NCORES),
                msg2_full[:].rearrange("r p b g -> p r b g"))

            # ---- layer 2 aggregation: t2T = sum_t (s_c*msg2)_t^T mT_t --
            t2 = [l2p.tile([P, 4, P], F32, tag=f"t2_{hh}", name=f"t2_{hh}")
                  for hh in range(2)]
            for t in range(JT):
                nc.vector.tensor_scalar_mul(
                    msg2_all[:, t, :], msg2_all[:, t, :], s_c[:, t:t + 1])
                for hh in range(2):
                    nc.tensor.matmul(
                        t2[hh][:], msg2_all[:, t, :],
                        mT[:, hh * 4:(hh + 1) * 4, t, :],
                        start=(t == 0), stop=(t == JT - 1))

            # ---- h2 = srb * t2T; segment max; classifier ---------------
            h2s = late.tile([P, IB, P], F32, tag="h2s", name="h2s")
            pooledT = late.tile([P, G_LOCAL], F32, tag="pooledT", name="pooledT")
            out_sb = late.tile([G_LOCAL, C], F32, tag="out_sb", name="out_sb")
            for hh in range(2):
                nc.vector.scalar_tensor_tensor(
                    out=h2s[:, hh * 4:(hh + 1) * 4, :], in0=t2[hh][:],
                    scalar=1.0, in1=srb[:, hh * 4:(hh + 1) * 4, :],
                    op0=BP, op1=MUL)
                nc.vector.reduce_max(
                    out=pooledT[:, hh * 4:(hh + 1) * 4],
                    in_=h2s[:, hh * 4:(hh + 1) * 4, :], axis=AX)
            with tc.tile_pool(name="clspsum", bufs=1, space="PSUM") as clsp:
                cps = clsp.tile([G_LOCAL, C], F32, tag="cls", name="cls")
                nc.tensor.matmul(cps[:], pooledT[:], wcT[:],
                                 start=True, stop=False)
                nc.tensor.matmul(cps[:], ones_row8[:], bc_row[:],
                                 start=False, stop=True)
                nc.vector.tensor_copy(out_sb[:], cps[:])
            nc.sync.dma_start(out_l.ap(), out_sb[:])

    nc.compile()
    return nc


def _get_nc():
    if "nc" not in _CACHE:
        _CACHE["nc"] = _build()
    return _CACHE["nc"]


def kernel(**inputs):
    m = np.ascontiguousarray(np.asarray(inputs["m"], dtype=np.float32))
    x = np.ascontiguousarray(np.asarray(inputs["x"]).astype(np.int32))
    emb = np.ascontiguousarray(np.asarray(inputs["emb"], dtype=np.float32))
    w1 = np.ascontiguousarray(np.asarray(inputs["w1"], dtype=np.float32))
    b1 = np.ascontiguousarray(np.asarray(inputs["b1"], dtype=np.float32))
    w2 = np.ascontiguousarray(np.asarray(inputs["w2"], dtype=np.float32))
    b2 = np.ascontiguousarray(np.asarray(inputs["b2"], dtype=np.float32))
    wc = np.ascontiguousarray(np.asarray(inputs["wc"], dtype=np.float32))
    bc = np.ascontiguousarray(np.asarray(inputs["bc"], dtype=np.float32))

    nc = _get_nc()
    in_maps = []
    for k in range(NCORES):
        in_maps.append({
            "m_shard": np.ascontiguousarray(m[k * NS:(k + 1) * NS]),
            "x_in": x, "emb_in": emb,
            "w1_in": w1, "b1_in": b1, "w2_in": w2, "b2_in": b2,
            "wc_in": wc, "bc_in": bc,
        })
    res = bass_utils.run_bass_kernel_spmd(
        nc, in_maps, core_ids=list(range(NCORES)))
    out = np.concatenate([res.results[k]["out_l"] for k in range(NCORES)], axis=0)
    return out.astype(np.float32)
